# revision 2
# baseline (speedup 1.0000x reference)
"""Trainium2 Bass kernel for nn_DECSeq3 (DynamicEdgeConv over streamlines), v2.

Self-contained: hardcodes shapes from the problem spec.
  pos [131072, 3] f32, edge_index [2, 245760] int64, plus MLP weights.
  Output [8192, 2] f32.

Data-parallel over 8192 streamlines across 8 NeuronCores (1024 per core).

v2 changes vs v1 baseline:
  - fw/bw stage-1 inputs are identical (edge construction makes the flipped
    backward input equal the forward one) -> single xe input, half the DMA.
  - kNN top-5 runs on a full 128-wide masked distance row (constant
    block-diagonal mask baked on host) -> no small diag-extraction DMAs,
    indices come out block-local.
  - neighbor gather/max is fully on-chip: one idx transpose per tile, then
    per 128-node block 5 K=1 broadcast matmuls + one DVE is_equal build the
    one-hot gather matrix in [q, (k,p)] orientation; a bf16 gather matmul
    against node-major B yields the gathered features feature-major.  No HBM
    staging, no indirect DMA.
  - B matmul and gather matmuls in bf16 (1 cyc/col on PE vs 4 for fp32).
    Distances and stage-1 stay fp32: the top-5 selection flips neighbors
    and blows the error budget with anything less.
"""

import os
import sys

if "/opt/trn_rl_repo" not in sys.path:
    sys.path.insert(0, "/opt/trn_rl_repo")

import numpy as np
import ml_dtypes

# ---------------- problem constants ----------------
B_FULL = 8192
L = 16
D = 3
K = 5
NCLS = 2
P = L - 1          # 15 real points per streamline
PP = 16            # padded points
EPS = 1e-5

NCORES = 8
BC = 1024          # streamlines per core
NODES = BC * PP    # 16384 padded nodes per core
NTILES = 8
TNODES = NODES // NTILES      # 2048 nodes per tile
TSTRL = BC // NTILES          # 128 streamlines per tile
NBLK = TNODES // 128          # 16 blocks of 128 nodes per tile
BIG_NEG = -1.0e30

_CACHE = {}


# ---------------- device program ----------------
def _build_program():
    import concourse.bacc as bacc
    import concourse.bass as bass
    import concourse.mybir as mybir
    from concourse.tile import TileContext
    from concourse.masks import make_identity

    dt = mybir.dt
    f32 = dt.float32
    f32r = dt.float32r
    bf16 = dt.bfloat16
    u16 = dt.uint16
    AF = mybir.ActivationFunctionType
    OP = mybir.AluOpType
    AX = mybir.AxisListType

    nc = bacc.Bacc("TRN2", target_bir_lowering=False)

    # ---- DRAM I/O ----
    xef = nc.dram_tensor("xef", [7, NODES], f32, kind="ExternalInput")
    s1wf = nc.dram_tensor("s1wf", [7, 64], f32, kind="ExternalInput")
    s1wb = nc.dram_tensor("s1wb", [7, 64], f32, kind="ExternalInput")
    s1g = nc.dram_tensor("s1g", [64, 1], f32, kind="ExternalInput")
    s1b = nc.dram_tensor("s1b", [64, 1], f32, kind="ExternalInput")
    wa = nc.dram_tensor("wa", [65, 128], f32r, kind="ExternalInput")
    wdt = nc.dram_tensor("wdt", [64, 128], f32, kind="ExternalInput")
    wl1x1 = nc.dram_tensor("wl1x1", [65, 1024], f32r, kind="ExternalInput")
    wl1x2 = nc.dram_tensor("wl1x2", [128, 1024], f32r, kind="ExternalInput")
    wm1 = nc.dram_tensor("wm1", [128, 8 * 512], f32r, kind="ExternalInput")
    bm1 = nc.dram_tensor("bm1", [1, 512], f32r, kind="ExternalInput")
    wm2 = nc.dram_tensor("wm2", [128, 4 * 256], f32r, kind="ExternalInput")
    bm2 = nc.dram_tensor("bm2", [1, 256], f32r, kind="ExternalInput")
    wm3 = nc.dram_tensor("wm3", [128, 2 * 2], f32r, kind="ExternalInput")
    bm3 = nc.dram_tensor("bm3", [1, 2], f32r, kind="ExternalInput")
    maskc = nc.dram_tensor("maskc", [128, 128], f32, kind="ExternalInput")
    iotap = nc.dram_tensor("iotap", [128, 1], f32, kind="ExternalInput")
    allon = nc.dram_tensor("allon", [128, 128], bf16, kind="ExternalInput")
    onesr = nc.dram_tensor("onesr", [1, BC], f32r, kind="ExternalInput")
    out_t = nc.dram_tensor("out", [2, BC], f32, kind="ExternalOutput")
    DBG = os.environ.get("KDEBUG", "") == "1"
    if DBG:
        dbg_x1 = nc.dram_tensor("dbg_x1", [128, TNODES], f32, kind="ExternalOutput")
        dbg_idx = nc.dram_tensor("dbg_idx", [128, 128], dt.uint16, kind="ExternalOutput")
        dbg_x2 = nc.dram_tensor("dbg_x2", [128, TNODES], bf16, kind="ExternalOutput")
        dbg_b = nc.dram_tensor("dbg_b", [128, 1024], bf16, kind="ExternalOutput")
        dbg_oh = nc.dram_tensor("dbg_oh", [128, 640], bf16, kind="ExternalOutput")
        dbg_pl8 = [nc.dram_tensor(f"dbg_pl{mm_}", [128, BC], f32r,
                                  kind="ExternalOutput") for mm_ in range(8)]
        dbg_x1t = nc.dram_tensor("dbg_x1t", [65, TNODES], bf16, kind="ExternalOutput")

    with TileContext(nc) as tc:
        with tc.tile_pool(name="const", bufs=1) as cpool, \
             tc.tile_pool(name="wpool", bufs=1) as wpool, \
             tc.tile_pool(name="pooled", bufs=1) as plpool, \
             tc.tile_pool(name="head", bufs=1) as headp:

            ident = cpool.tile([128, 128], f32)
            make_identity(nc, ident[:])
            ones_row = cpool.tile([1, BC], f32r)
            nc.sync.dma_start(out=ones_row[:], in_=onesr[:])

            t_s1wf = wpool.tile([7, 64], f32, name="t_s1wf")
            t_s1wb = wpool.tile([7, 64], f32, name="t_s1wb")
            nc.sync.dma_start(out=t_s1wf[:], in_=s1wf[:])
            nc.sync.dma_start(out=t_s1wb[:], in_=s1wb[:])
            t_s1g = wpool.tile([64, 1], f32)
            nc.sync.dma_start(out=t_s1g[:], in_=s1g[:])
            t_s1b = wpool.tile([64, 1], f32)
            nc.sync.dma_start(out=t_s1b[:], in_=s1b[:])
            t_wa = wpool.tile([65, 128], f32r)
            nc.sync.dma_start(out=t_wa[:], in_=wa[:])
            t_wdt = wpool.tile([64, 128], f32)
            nc.sync.dma_start(out=t_wdt[:], in_=wdt[:])
            t_wl1x1 = wpool.tile([65, 1024], f32r)
            nc.sync.dma_start(out=t_wl1x1[:], in_=wl1x1[:])
            t_wl1x2 = wpool.tile([128, 1024], f32r)
            nc.sync.dma_start(out=t_wl1x2[:], in_=wl1x2[:])
            t_wm1 = wpool.tile([128, 8 * 512], f32r)
            nc.sync.dma_start(out=t_wm1[:], in_=wm1[:])
            t_bm1 = wpool.tile([1, 512], f32r)
            nc.sync.dma_start(out=t_bm1[:], in_=bm1[:])
            t_wm2 = wpool.tile([128, 4 * 256], f32r)
            nc.sync.dma_start(out=t_wm2[:], in_=wm2[:])
            t_bm2 = wpool.tile([1, 256], f32r)
            nc.sync.dma_start(out=t_bm2[:], in_=bm2[:])
            t_wm3 = wpool.tile([128, 4], f32r)
            nc.sync.dma_start(out=t_wm3[:], in_=wm3[:])
            t_bm3 = wpool.tile([1, 2], f32r)
            nc.sync.dma_start(out=t_bm3[:], in_=bm3[:])
            t_mask = wpool.tile([128, 128], f32)
            nc.sync.dma_start(out=t_mask[:], in_=maskc[:])
            t_iota = wpool.tile([128, 1], f32)
            nc.sync.dma_start(out=t_iota[:], in_=iotap[:])
            t_ones = wpool.tile([128, 128], bf16)
            nc.sync.dma_start(out=t_ones[:], in_=allon[:])

            # pooled pre-activations, one [128, BC] buffer per 128-ch chunk
            pooled = [plpool.tile([128, BC], f32r, name=f"pooled{m}",
                                  tag=f"pooled{m}") for m in range(8)]

            with tc.tile_pool(name="io", bufs=2) as iop, \
                 tc.tile_pool(name="s1st", bufs=2) as s1st, \
                 tc.tile_pool(name="xt", bufs=2) as xtp, \
                 tc.tile_pool(name="knn", bufs=2) as knnp, \
                 tc.tile_pool(name="bst", bufs=2) as bstp, \
                 tc.tile_pool(name="ps_mix", bufs=2, space="PSUM") as ps_mix, \
                 tc.tile_pool(name="ps_big", bufs=2, space="PSUM") as ps_big:

                ST = {}

                def ph1(t):
                    c0 = t * TNODES
                    x1g = xtp.tile([128, TNODES], f32, tag="x1g", name=f"x1g{t}")
                    x1r2 = xtp.tile([128, TNODES], f32, tag="x1r2", name=f"x1r2{t}")
                    x2t = xtp.tile([128, TNODES], f32r, tag="x2t", name=f"x2t{t}")
                    x1t = xtp.tile([65, TNODES], f32r, tag="x1t", name=f"x1t{t}")
                    ST[t] = dict(x1g=x1g, x1r2=x1r2, x1t=x1t, x2t=x2t)
                    nc.gpsimd.memset(x1g[64:128, :], -1.0)

                    for ch in range(TNODES // 1024):
                        dl = slice(ch * 1024, (ch + 1) * 1024)
                        xec = iop.tile([7, 1024], f32, tag="xec")
                        nc.sync.dma_start(
                            out=xec[:], in_=xef[:, c0 + ch * 1024:c0 + (ch + 1) * 1024])
                        pft = ps_mix.tile([128, 1024], f32, tag="mix")
                        pf = pft[0:64, :]
                        for h in range(2):
                            nc.tensor.matmul(
                                out=pf[:, h * 512:(h + 1) * 512], lhsT=t_s1wf[:],
                                rhs=xec[:, h * 512:(h + 1) * 512],
                                start=True, stop=True)
                        fwat = s1st.tile([64, TNODES], f32, tag="fwa")
                        fwa = fwat[:, 0:1024]
                        nc.scalar.activation(out=fwa, in_=pf[:, :], func=AF.Relu,
                                             bias=t_s1b[:], scale=t_s1g[:])
                        pbt = ps_mix.tile([128, 1024], f32, tag="mix")
                        pb = pbt[0:64, :]
                        for h in range(2):
                            nc.tensor.matmul(
                                out=pb[:, h * 512:(h + 1) * 512], lhsT=t_s1wb[:],
                                rhs=xec[:, h * 512:(h + 1) * 512],
                                start=True, stop=True)
                        nc.scalar.activation(out=pb[:, :], in_=pb[:, :], func=AF.Relu,
                                             bias=t_s1b[:], scale=t_s1g[:])
                        nc.vector.tensor_tensor(out=x1g[0:64, dl], in0=fwa,
                                                in1=pb[:, :], op=OP.add)

                def ph2(t):
                    x1g, x1r2 = ST[t]["x1g"], ST[t]["x1r2"]
                    x1t = ST[t]["x1t"]
                    nc.scalar.copy(out=x1t[:], in_=x1g[0:65, :])
                    nc.vector.tensor_scalar_mul(out=x1r2[0:64, :],
                                                in0=x1g[0:64, :], scalar1=2.0)
                    sq64 = s1st.tile([64, TNODES], f32, tag="fwa")
                    nc.gpsimd.tensor_mul(out=sq64[:], in0=x1g[0:64, :],
                                         in1=x1g[0:64, :])
                    nc.sync.dma_start(out=x1r2[64:128, :], in_=sq64[:])

                def ph3(t):
                    x1g, x1r2 = ST[t]["x1g"], ST[t]["x1r2"]
                    idxf = knnp.tile([128, 128], u16, tag="idxf", name=f"idxf{t}")
                    for r in range(2):
                        pd = ps_big.tile([128, 1024], f32, tag="big")
                        for n in range(8):
                            nt = r * 8 + n
                            sl = slice(nt * 128, (nt + 1) * 128)
                            nc.tensor.matmul(out=pd[:, n * 128:(n + 1) * 128],
                                             lhsT=x1g[:, sl], rhs=x1r2[:, sl],
                                             start=True, stop=True)
                        dal = ST[t]["x1r2"][:, r * 1024:(r + 1) * 1024]
                        nc.vector.tensor_tensor(
                            out=dal.rearrange("p (n q) -> p n q", n=8),
                            in0=pd[:].rearrange("p (n q) -> p n q", n=8),
                            in1=t_mask[:].unsqueeze(1).to_broadcast([128, 8, 128]),
                            op=OP.add)
                        for n in range(8):
                            g = r * 8 + n
                            m8f = knnp.tile([128, 8], f32, tag="m8f")
                            nc.vector.max(out=m8f[:],
                                          in_=dal[:, n * 128:(n + 1) * 128])
                            nc.vector.max_index(
                                out=idxf[:, g * 8:(g + 1) * 8],
                                in_max=m8f[:],
                                in_values=dal[:, n * 128:(n + 1) * 128])
                    # transpose idx matrix once per tile: [p, (blk,k)] -> [(blk,k), p]
                    idxf32 = knnp.tile([128, 128], f32, tag="idxf32")
                    nc.vector.tensor_copy(out=idxf32[:], in_=idxf[:])
                    pTt = ps_mix.tile([128, 1024], f32, tag="mix")
                    pT = pTt[:, 0:128]
                    nc.tensor.transpose(out=pT, in_=idxf32[:], identity=ident[:])
                    idxT = knnp.tile([128, 128], bf16, tag="idxT", name=f"idxT{t}")
                    nc.scalar.copy(out=idxT[:], in_=pT)
                    ST[t]["idxT"] = idxT
                    if DBG and t == 0:
                        nc.sync.dma_start(out=dbg_x1[:], in_=x1g[:])
                        nc.sync.dma_start(out=dbg_idx[:], in_=idxf[:])
                        nc.sync.dma_start(out=dbg_x1t[:], in_=ST[t]["x1t"][:])

                def ph45(t):
                    x1g, x1t, x2t = ST[t]["x1g"], ST[t]["x1t"], ST[t]["x2t"]
                    idxT = ST[t]["idxT"]
                    for r in range(2):
                        # B = Wd^T x1 for 8 blocks, node-major [q, c]
                        pb8 = ps_big.tile([128, 1024], f32, tag="big")
                        for n in range(8):
                            nt = r * 8 + n
                            sl = slice(nt * 128, (nt + 1) * 128)
                            nc.tensor.matmul(out=pb8[:, n * 128:(n + 1) * 128],
                                             lhsT=x1g[0:64, sl], rhs=t_wdt[:],
                                             start=True, stop=True)
                        b_sb = bstp.tile([128, 1024], f32r, tag="bsb")
                        nc.scalar.copy(out=b_sb[:], in_=pb8[:])
                        if DBG and t == 0 and r == 0:
                            nc.sync.dma_start(out=dbg_b[:], in_=b_sb[:])
                        # A part for this r-group
                        psA = ps_mix.tile([128, 1024], f32, tag="mix")
                        for h in range(2):
                            gl = slice(r * 1024 + h * 512, r * 1024 + (h + 1) * 512)
                            nc.tensor.matmul(out=psA[:, h * 512:(h + 1) * 512],
                                             lhsT=t_wa[:], rhs=x1t[:, gl],
                                             start=True, stop=True)
                        for n in range(8):
                            nt = r * 8 + n
                            # this block's 5 index rows flattened onto partition 0
                            idxR = knnp.tile([1, K * 128], bf16, tag="idxR")
                            eng = nc.scalar if n % 2 == 0 else nc.sync
                            eng.dma_start(out=idxR[:],
                                          in_=idxT[nt * 8:nt * 8 + K, :])
                            psQ = ps_big.tile([128, 1024], f32, tag="big")
                            q0 = 0
                            nc.tensor.matmul(
                                out=psQ[:, 0:512],
                                lhsT=t_ones[0:1, :],
                                rhs=idxR[0:1, q0:q0 + 512],
                                start=True, stop=True)
                            nc.tensor.matmul(
                                out=psQ[:, 512:640],
                                lhsT=t_ones[0:1, :],
                                rhs=idxR[0:1, q0 + 512:q0 + 640],
                                start=True, stop=True)
                            ohT = knnp.tile([128, 640], f32r, tag="ohT")
                            nc.vector.tensor_tensor(
                                out=ohT[:], in0=psQ[:, 0:640],
                                in1=t_iota[:].to_broadcast([128, 640]),
                                op=OP.is_equal)
                            if DBG and t == 0 and r == 0 and n == 0:
                                nc.sync.dma_start(out=dbg_oh[:], in_=ohT[:])
                            psG = ps_big.tile([128, 1024], f32, tag="big")
                            nc.tensor.matmul(out=psG[:, 0:512],
                                             lhsT=b_sb[:, n * 128:(n + 1) * 128],
                                             rhs=ohT[:, 0:512],
                                             start=True, stop=True)
                            nc.tensor.matmul(out=psG[:, 512:640],
                                             lhsT=b_sb[:, n * 128:(n + 1) * 128],
                                             rhs=ohT[:, 512:640],
                                             start=True, stop=True)
                            bl = slice(nt * 128, (nt + 1) * 128)
                            xb_ = x2t[:, bl]
                            nc.vector.tensor_copy(out=xb_, in_=psG[:, 0:128])
                            nc.vector.tensor_tensor(out=xb_, in0=xb_,
                                                    in1=psG[:, 128:256], op=OP.max)
                            nc.vector.tensor_tensor(out=xb_, in0=xb_,
                                                    in1=psG[:, 256:384], op=OP.max)
                            nc.vector.tensor_tensor(out=xb_, in0=xb_,
                                                    in1=psG[:, 384:512], op=OP.max)
                            nc.vector.tensor_tensor(out=xb_, in0=xb_,
                                                    in1=psG[:, 512:640], op=OP.max)
                            nc.vector.tensor_tensor(
                                out=xb_, in0=xb_,
                                in1=psA[:, n * 128:(n + 1) * 128], op=OP.add)
                            nc.scalar.activation(out=xb_, in_=xb_,
                                                 func=AF.Relu)

                def ph6(t):
                    x1t, x2t = ST[t]["x1t"], ST[t]["x2t"]
                    if DBG and t == 0:
                        nc.sync.dma_start(out=dbg_x2[:], in_=x2t[:])
                    for m in range(8):
                        for cc in range(TNODES // 1024):
                            pl1 = ps_big.tile([128, 1024], f32, tag="big")
                            for h in range(2):
                                sl = slice(cc * 1024 + h * 512,
                                           cc * 1024 + (h + 1) * 512)
                                osl = slice(h * 512, (h + 1) * 512)
                                nc.tensor.matmul(
                                    out=pl1[:, osl],
                                    lhsT=t_wl1x1[:, m * 128:(m + 1) * 128],
                                    rhs=x1t[:, sl],
                                    start=True, stop=False)
                                nc.tensor.matmul(
                                    out=pl1[:, osl],
                                    lhsT=t_wl1x2[:, m * 128:(m + 1) * 128],
                                    rhs=x2t[:, sl],
                                    start=False, stop=True)
                            pv = pl1[:].rearrange("p (s q) -> p s q", q=16)[:, :, 0:15]
                            psl = slice(t * TSTRL + cc * 64,
                                        t * TSTRL + (cc + 1) * 64)
                            nc.vector.tensor_reduce(out=pooled[m][:, psl], in_=pv,
                                                    axis=AX.X, op=OP.max)

                def whole_body():
                    for base in range(0, NTILES, 2):
                        for ph in (ph1, ph2, ph3, ph45, ph6):
                            ph(base)
                            ph(base + 1)
                        ST.pop(base); ST.pop(base + 1)

                def head_body():
                    h1 = pooled
                    if DBG:
                        for mm_ in range(8):
                            nc.sync.dma_start(out=dbg_pl8[mm_][:], in_=pooled[mm_][:])
                    for m in range(8):
                        nc.scalar.activation(out=h1[m][:], in_=h1[m][:], func=AF.Relu)
                    t1 = [headp.tile([128, BC], f32r, name=f"t1_{o}", tag=f"t1_{o}")
                          for o in range(4)]
                    wm1v = t_wm1[:].rearrange("p (a m) -> p a m", a=8)
                    for o in range(4):
                        pm1 = ps_big.tile([128, 1024], f32, tag="big")
                        for h in range(2):
                            osl = slice(h * 512, (h + 1) * 512)
                            for kc in range(8):
                                nc.tensor.matmul(
                                    out=pm1[:, osl],
                                    lhsT=wm1v[:, kc, o * 128:(o + 1) * 128],
                                    rhs=h1[kc][:, osl],
                                    start=(kc == 0), stop=False)
                            nc.tensor.matmul(
                                out=pm1[:, osl],
                                lhsT=t_bm1[:, o * 128:(o + 1) * 128],
                                rhs=ones_row[:, osl],
                                start=False, stop=True)
                        nc.scalar.activation(out=t1[o][:], in_=pm1[:], func=AF.Relu)
                    t2 = [headp.tile([128, BC], f32r, name=f"t2_{o}", tag=f"t2_{o}")
                          for o in range(2)]
                    wm2v = t_wm2[:].rearrange("p (a m) -> p a m", a=4)
                    for o in range(2):
                        pm2 = ps_big.tile([128, 1024], f32, tag="big")
                        for h in range(2):
                            osl = slice(h * 512, (h + 1) * 512)
                            for kc in range(4):
                                nc.tensor.matmul(
                                    out=pm2[:, osl],
                                    lhsT=wm2v[:, kc, o * 128:(o + 1) * 128],
                                    rhs=t1[kc][:, osl],
                                    start=(kc == 0), stop=False)
                            nc.tensor.matmul(
                                out=pm2[:, osl],
                                lhsT=t_bm2[:, o * 128:(o + 1) * 128],
                                rhs=ones_row[:, osl],
                                start=False, stop=True)
                        nc.scalar.activation(out=t2[o][:], in_=pm2[:], func=AF.Relu)
                    outs = headp.tile([2, BC], f32, tag="outs")
                    wm3v = t_wm3[:].rearrange("p (a m) -> p a m", a=2)
                    for h in range(2):
                        osl = slice(h * 512, (h + 1) * 512)
                        pm3t = ps_mix.tile([128, 1024], f32, tag="mix")
                        pm3 = pm3t[0:2, 0:512]
                        for kc in range(2):
                            nc.tensor.matmul(
                                out=pm3,
                                lhsT=wm3v[:, kc, :],
                                rhs=t2[kc][:, osl],
                                start=(kc == 0), stop=False)
                        nc.tensor.matmul(out=pm3,
                                         lhsT=t_bm3[:],
                                         rhs=ones_row[:, osl],
                                         start=False, stop=True)
                        nc.scalar.copy(out=outs[:, osl], in_=pm3)
                    nc.sync.dma_start(out=out_t[:], in_=outs[:])

                REPEAT = int(os.environ.get("KREPEAT", "1"))
                if REPEAT > 1:
                    with tc.For_i(0, REPEAT, 1):
                        whole_body()
                        head_body()
                else:
                    whole_body()
                    head_body()

    nc.finalize()
    return nc


# ---------------- host-side prep ----------------
def _prep_inputs(pos, edge_index,
                 W_c1fw, b_c1fw, W_c1bw, b_c1bw, g_bn1, be_bn1,
                 W_e, b_e, g_e, be_e,
                 W_l1, b_l1, g_l1, be_l1,
                 W_m1, b_m1, g_m1, be_m1,
                 W_m2, b_m2, g_m2, be_m2,
                 W_m3, b_m3):
    f = np.float32
    bf = ml_dtypes.bfloat16
    pos = np.asarray(pos, f)
    E = edge_index.shape[1]
    N = E // 2
    second = np.asarray(edge_index[:, N:])
    src, dst = second[0], second[1]
    xe = np.concatenate([pos[dst] - pos[src], pos[src]], axis=1).astype(f)
    # the reference feeds the REVERSED second edge half to both convs
    xe = xe.reshape(B_FULL, P, 2 * D)[::-1, ::-1, :]

    # [B, 15, 6] -> per-core feature-major [7, NODES] with ones row
    xpad = np.zeros((B_FULL, PP, 7), f)
    xpad[:, :P, :6] = xe
    xpad[:, :, 6] = 1.0
    xpad = xpad.reshape(NCORES, NODES, 7)
    xef = np.ascontiguousarray(xpad.transpose(0, 2, 1))

    sq = np.sqrt(np.asarray(1.0 + EPS, f))
    g1 = (np.asarray(g_bn1, f) / sq)[:, None]
    be1 = np.asarray(be_bn1, f)[:, None]
    s1wf = np.ascontiguousarray(
        np.concatenate([np.asarray(W_c1fw, f), np.asarray(b_c1fw, f)[:, None]], 1).T)
    s1wb = np.ascontiguousarray(
        np.concatenate([np.asarray(W_c1bw, f), np.asarray(b_c1bw, f)[:, None]], 1).T)

    W_e = np.asarray(W_e, f)
    Wi, Wd = W_e[:, :64], W_e[:, 64:]
    wa = np.ascontiguousarray(
        np.concatenate([(Wi - Wd).T, -np.asarray(b_e, f)[None, :]], 0))
    wdt = np.ascontiguousarray(Wd.T)

    ge = np.asarray(g_e, f) / sq
    bee = np.asarray(be_e, f)
    W_l1 = np.asarray(W_l1, f)
    Wl1x1 = W_l1[:, :64]
    Wl1x2 = W_l1[:, 64:] * ge[None, :]
    bl1 = np.asarray(b_l1, f) + W_l1[:, 64:] @ bee
    wl1x1 = np.ascontiguousarray(np.concatenate([Wl1x1.T, -bl1[None, :]], 0))
    wl1x2 = np.ascontiguousarray(Wl1x2.T)

    def m_fold(W, b, g_prev, be_prev, kchunks):
        W = np.asarray(W, f)
        gp = np.asarray(g_prev, f) / sq
        Wf = W * gp[None, :]
        bf_ = np.asarray(b, f) + W @ np.asarray(be_prev, f)
        lhsT = Wf.T
        Kd, Md = lhsT.shape
        arr = lhsT.reshape(kchunks, 128, Md).transpose(1, 0, 2).reshape(128, -1)
        return np.ascontiguousarray(arr), bf_[None, :]

    wm1a, bm1v = m_fold(W_m1, b_m1, g_l1, be_l1, 8)
    wm2a, bm2v = m_fold(W_m2, b_m2, g_m1, be_m1, 4)
    wm3a, bm3v = m_fold(W_m3, b_m3, g_m2, be_m2, 2)

    pidx = np.arange(128)
    qidx = np.arange(128)
    same_grp = (qidx[None, :] // 16) == (pidx[:, None] // 16)
    valid = same_grp & ((qidx[None, :] % 16) != 15)
    maskc = np.where(valid, 0.0, BIG_NEG).astype(f)
    iotap = np.arange(128, dtype=f)[:, None]
    allon = np.ones((128, 128), bf)

    shared = {
        "s1wf": s1wf, "s1wb": s1wb, "s1g": g1, "s1b": be1,
        "wa": wa, "wdt": wdt,
        "wl1x1": wl1x1, "wl1x2": wl1x2,
        "wm1": wm1a, "bm1": bm1v,
        "wm2": wm2a, "bm2": bm2v,
        "wm3": wm3a, "bm3": bm3v,
        "maskc": maskc, "iotap": iotap, "allon": allon,
        "onesr": np.ones((1, BC), f),
    }
    in_maps = []
    for c in range(NCORES):
        m = dict(shared)
        m["xef"] = xef[c]
        in_maps.append(m)
    return in_maps


def _get_runner():
    if "runner" in _CACHE:
        return _CACHE["runner"]
    from concourse.bass_utils import run_bass_kernel_spmd
    nc = _build_program()
    _CACHE["nc"] = nc

    def runner(in_maps):
        return run_bass_kernel_spmd(nc, in_maps, list(range(NCORES))).results

    _CACHE["runner"] = runner
    return runner


def kernel(**inputs):
    in_maps = _prep_inputs(**inputs)
    results = _get_runner()(in_maps)
    out = np.empty((B_FULL, NCLS), np.float32)
    for c in range(NCORES):
        out[c * BC:(c + 1) * BC, :] = results[c]["out"].T
    return out
